# revision 1
# baseline (speedup 1.0000x reference)
"""PointNet++ backbone decoder on 8 TRN2 NeuronCores.

Data parallel over batch B=8: one point cloud per core (per the sharding
hint). Host numpy computes only the coordinate-derived index structures
(FPS order, ball-query neighbor lists, 3-NN interpolation weights) and
packs them as dense gather/interpolation matrices; every dense FLOP of
the module (all shared-MLP layers, neighborhood gathers, max-pools and
the three feature-propagation interpolations) runs on-device in a single
NEFF per core, expressed as channel-major matmul chains on the PE array.

Channel-major convention: activations live as [C(part), pts(free)] SBUF
tiles so each MLP layer is psum[Cout,pts] += wT_chunk[Cin_chunk,Cout] x
act[Cin_chunk,pts] with no transposes between layers. Gathers and 3-NN
interpolation are matmuls against host-built selection matrices, which
needs the source features point-major [pts, C]; those small tensors are
transposed on the PE via identity matmuls.
"""

import sys

for _p in ("/opt/trn_rl_repo", "/root/.axon_site/_ro/trn_rl_repo"):
    if _p not in sys.path:
        sys.path.insert(0, _p)

import numpy as np

import concourse.bass as bass
import concourse.bacc as bacc
import concourse.tile as tile
import concourse.mybir as mybir
from concourse.bass_utils import run_bass_kernel_spmd

FP32 = mybir.dt.float32
ActF = mybir.ActivationFunctionType
P = 128

B, N = 8, 20000
N1, N2, N3 = 512, 256, 128  # centers per SA stage
K1, K2, K3 = 32, 16, 16
R1, R2, R3 = 0.4, 0.8, 1.2


# ---------------------------------------------------------------- host math
def _sqdist(a, b):
    # (M,3),(n,3) -> (M,n) fp32, same formula as the reference's sqdist
    aa = (a * a).sum(-1).astype(np.float32)
    bb = (b * b).sum(-1).astype(np.float32)
    return aa[:, None] + bb[None, :] - np.float32(2.0) * (a @ b.T)


def _fps(xyz, npoint):
    n = xyz.shape[0]
    dists = np.full(n, 1e10, np.float32)
    idx = np.zeros(npoint, np.int64)
    last = 0
    for i in range(1, npoint):
        d = ((xyz - xyz[last]) ** 2).sum(-1)
        np.minimum(dists, d, out=dists)
        last = int(np.argmax(dists))
        idx[i] = last
    return idx


def _ball(new_xyz, xyz, radius, K):
    d2 = _sqdist(new_xyz, xyz)
    mask = d2 < np.float32(radius * radius)
    S = new_xyz.shape[0]
    idx = np.empty((S, K), np.int64)
    for i in range(S):
        w = np.flatnonzero(mask[i])
        k = min(len(w), K)
        idx[i, :k] = w[:k]
        if k < K:
            idx[i, k:] = w[0]
    return idx


def _three_nn(q, src):
    d2 = _sqdist(q, src)
    idx = np.argsort(d2, axis=1, kind="stable")[:, :3]
    d3 = np.take_along_axis(d2, idx, 1)
    recip = np.float32(1.0) / (d3 + np.float32(1e-8))
    w = recip / recip.sum(1, keepdims=True)
    return idx, w.astype(np.float32)


def _interp_T(idx, w, n_src):
    # rows=source points, cols=targets: out[s, n] = weight of src s for target n
    n_tgt = idx.shape[0]
    M = np.zeros((n_src, n_tgt), np.float32)
    cols = np.repeat(np.arange(n_tgt), 3)
    np.add.at(M, (idx.ravel(), cols), w.ravel())
    return M


def _onehot_T(idx, n_src):
    # (S,K) gather indices -> (n_src, S*K) one-hot selection matrix
    flat = idx.ravel()
    M = np.zeros((n_src, flat.size), np.float32)
    M[flat, np.arange(flat.size)] = np.float32(1.0)
    return M


def _host_maps(pc, weights):
    """Per-batch index/selection tensors. pc: (N, 4) fp32."""
    xyz = np.ascontiguousarray(pc[:, 0:3])
    f0 = np.ascontiguousarray(pc[:, 3:4])

    i1 = _fps(xyz, N1)
    x1 = xyz[i1]                       # (512,3)
    b1 = _ball(x1, xyz, R1, K1)        # (512,32)
    g1 = np.concatenate(
        [(xyz[b1] - x1[:, None, :]) / np.float32(R1), f0[b1]], -1
    )                                  # (512,32,4)
    g1T = np.ascontiguousarray(g1.reshape(N1 * K1, 4).T)

    i2 = _fps(x1, N2)
    x2 = x1[i2]
    b2 = _ball(x2, x1, R2, K2)         # (256,16)
    x2T = np.ascontiguousarray(
        ((x1[b2] - x2[:, None, :]) / np.float32(R2)).reshape(N2 * K2, 3).T
    )
    G2T = _onehot_T(b2, N1)            # (512, 4096)

    i3 = _fps(x2, N3)
    x3 = x2[i3]
    b3 = _ball(x3, x2, R3, K3)         # (128,16)
    x3T = np.ascontiguousarray(
        ((x2[b3] - x3[:, None, :]) / np.float32(R3)).reshape(N3 * K3, 3).T
    )
    G3T = _onehot_T(b3, N2)            # (256, 2048)

    ia, wa = _three_nn(x2, x3)
    W1T = _interp_T(ia, wa, N3)        # (128, 256)
    ib, wb = _three_nn(x1, x2)
    W2T = _interp_T(ib, wb, N2)        # (256, 512)
    ic, wc = _three_nn(xyz, x1)
    W3T = _interp_T(ic, wc, N1)        # (512, 20000)

    m = {
        "g1T": g1T, "x2T": x2T, "G2T": G2T, "x3T": x3T, "G3T": G3T,
        "W1T": W1T, "W2T": W2T, "W3T": np.ascontiguousarray(W3T),
        "f0T": np.ascontiguousarray(f0.T),
        "ident": np.eye(P, dtype=np.float32),
    }
    m.update(weights)
    return m


# ------------------------------------------------------------- device build
WSPECS = {
    "sa1_w0": (4, 128, [4]), "sa1_w1": (128, 128, [128]), "sa1_w2": (128, 256, [128]),
    "sa2_w0": (259, 128, [3, 128, 128]), "sa2_w1": (128, 128, [128]), "sa2_w2": (128, 256, [128]),
    "sa3_w0": (259, 128, [3, 128, 128]), "sa3_w1": (128, 128, [128]), "sa3_w2": (128, 256, [128]),
    "fp1_w0": (512, 256, [128] * 4), "fp1_w1": (256, 256, [128, 128]),
    "fp2_w0": (512, 256, [128] * 4), "fp2_w1": (256, 256, [128, 128]),
    "fp3_w0": (257, 256, [128, 128, 1]), "fp3_w1": (256, 256, [128, 128]),
}


def build_nc():
    nc = bacc.Bacc(None, target_bir_lowering=False, debug=False)
    dr = {}

    def din(name, shape):
        dr[name] = nc.dram_tensor(name, list(shape), FP32, kind="ExternalInput")
        return dr[name]

    g1T = din("g1T", (4, N1 * K1))
    x2T = din("x2T", (3, N2 * K2))
    G2T = din("G2T", (N1, N2 * K2))
    x3T = din("x3T", (3, N3 * K3))
    G3T = din("G3T", (N2, N3 * K3))
    W1T_d = din("W1T", (N3, N2))
    W2T_d = din("W2T", (N2, N1))
    W3T_d = din("W3T", (N1, N))
    f0T = din("f0T", (1, N))
    ident_d = din("ident", (P, P))
    for nm, (r, c, _) in WSPECS.items():
        din(nm, (r, c))
    outT = nc.dram_tensor("out_cm", [256, N], FP32, kind="ExternalOutput")

    with tile.TileContext(nc) as tc:
        with (
            tc.tile_pool(name="wp", bufs=1) as wp,
            tc.tile_pool(name="pp", bufs=1) as pp,
            tc.tile_pool(name="ac", bufs=2) as ac,
            tc.tile_pool(name="io", bufs=2) as io,
            tc.tile_pool(name="ps", bufs=4, space=bass.MemorySpace.PSUM) as ps,
            tc.tile_pool(name="pq", bufs=2, space=bass.MemorySpace.PSUM) as pq,
        ):
            W = {}
            for nm, (r, c, chunks) in WSPECS.items():
                tiles, r0 = [], 0
                for ch in chunks:
                    t = wp.tile([ch, c], FP32, tag=f"{nm}_{r0}")
                    nc.sync.dma_start(t[:], dr[nm][r0:r0 + ch, :])
                    tiles.append(t)
                    r0 += ch
                W[nm] = tiles
            ident = wp.tile([P, P], FP32, tag="ident")
            nc.sync.dma_start(ident[:], ident_d[:])
            w1t = wp.tile([N3, N2], FP32, tag="w1t")
            nc.sync.dma_start(w1t[:], W1T_d[:])
            w2t = [wp.tile([P, N1], FP32, tag=f"w2t{k}", name=f"w2t{k}") for k in range(2)]
            for k in range(2):
                nc.sync.dma_start(w2t[k][:], W2T_d[k * P:(k + 1) * P, :])

            def mm(pap, pairs):
                for i, (l, r) in enumerate(pairs):
                    nc.tensor.matmul(pap, l, r,
                                     start=(i == 0), stop=(i == len(pairs) - 1))

            def relu(dst, src):
                nc.scalar.activation(dst, src, ActF.Relu)

            def sa_stage(nm, n_pts, K, in_fn, f_cm):
                n_strips = n_pts // 512
                S = 512 // K
                w1_, w2_ = W[f"{nm}_w1"], W[f"{nm}_w2"]
                for s in range(n_strips):
                    pairs0 = in_fn(s)
                    p0 = ps.tile([P, 512], FP32, tag="pmm")
                    mm(p0[:], pairs0)
                    s0 = ac.tile([P, 512], FP32, tag="s0")
                    relu(s0[:], p0[:])
                    p1 = ps.tile([P, 512], FP32, tag="pmm")
                    mm(p1[:], [(w1_[0][:], s0[:])])
                    s1 = ac.tile([P, 512], FP32, tag="s1")
                    relu(s1[:], p1[:])
                    for h in range(2):
                        p2 = ps.tile([P, 512], FP32, tag="pmm")
                        mm(p2[:], [(w2_[0][:, h * P:(h + 1) * P], s1[:])])
                        s2 = ac.tile([P, 512], FP32, tag="s2")
                        relu(s2[:], p2[:])
                        nc.vector.tensor_reduce(
                            out=f_cm[h][:, s * S:(s + 1) * S],
                            in_=s2[:].rearrange("p (s k) -> p s k", k=K),
                            axis=mybir.AxisListType.X,
                            op=mybir.AluOpType.max,
                        )

            def in_sa1(s):
                g = io.tile([4, 512], FP32, tag="g1")
                nc.sync.dma_start(g[:], g1T[:, s * 512:(s + 1) * 512])
                return [(W["sa1_w0"][0][:], g[:])]

            def mk_in(GT, xT, src_pm, w0, nchunks, tagp):
                def f(s):
                    xt = io.tile([3, 512], FP32, tag="xt")
                    nc.sync.dma_start(xt[:], xT[:, s * 512:(s + 1) * 512])
                    Gts = []
                    for kc in range(nchunks):
                        Gt = io.tile([P, 512], FP32, tag=f"G{kc}")
                        nc.sync.dma_start(
                            Gt[:], GT[kc * P:(kc + 1) * P, s * 512:(s + 1) * 512])
                        Gts.append(Gt)
                    gath = []
                    for h in range(2):
                        pg = pq.tile([P, 512], FP32, tag="pg")
                        mm(pg[:], [(src_pm[kc][:, h * P:(h + 1) * P], Gts[kc][:])
                                   for kc in range(nchunks)])
                        gt = ac.tile([P, 512], FP32, tag=f"gath{h}")
                        nc.vector.tensor_copy(gt[:], pg[:])
                        gath.append(gt)
                    return [(w0[0][:], xt[:]), (w0[1][:], gath[0][:]),
                            (w0[2][:], gath[1][:])]
                return f

            def to_pm(f_cm, n_centers, tagp):
                pm = []
                for t in range(n_centers // P):
                    pt = pp.tile([P, 256], FP32, tag=f"{tagp}{t}")
                    for h in range(2):
                        tps = pq.tile([P, P], FP32, tag="ptps")
                        nc.tensor.transpose(
                            tps[:], f_cm[h][:, t * P:(t + 1) * P], ident[:])
                        nc.vector.tensor_copy(pt[:, h * P:(h + 1) * P], tps[:])
                    pm.append(pt)
                return pm

            # ---- SA1
            f1_cm = [pp.tile([P, N1], FP32, tag=f"f1cm{h}", name=f"f1cm{h}") for h in range(2)]
            sa_stage("sa1", N1 * K1, K1, in_sa1, f1_cm)
            f1_pm = to_pm(f1_cm, N1, "f1pm")

            # ---- SA2
            f2_cm = [pp.tile([P, N2], FP32, tag=f"f2cm{h}", name=f"f2cm{h}") for h in range(2)]
            sa_stage("sa2", N2 * K2, K2,
                     mk_in(G2T, x2T, f1_pm, W["sa2_w0"], 4, "s2"), f2_cm)
            f2_pm = to_pm(f2_cm, N2, "f2pm")

            # ---- SA3
            f3_cm = [pp.tile([P, N3], FP32, tag=f"f3cm{h}", name=f"f3cm{h}") for h in range(2)]
            sa_stage("sa3", N3 * K3, K3,
                     mk_in(G3T, x3T, f2_pm, W["sa3_w0"], 2, "s3"), f3_cm)
            f3_pm = to_pm(f3_cm, N3, "f3pm")

            def fp_block(w0, w1_, icm, skip, ncols, out_cb):
                # L0: Cin=512 (interp 0:256, skip 256:512), L1: 256->256
                l0 = []
                for h in range(2):
                    pl = ps.tile([P, ncols], FP32, tag="pmm")
                    mm(pl[:], [(w0[kc][:, h * P:(h + 1) * P], rhs[:])
                               for kc, rhs in enumerate(
                                   [icm[0][:], icm[1][:], skip[0][:], skip[1][:]])])
                    t = ac.tile([P, ncols], FP32, tag=f"fpl0_{h}")
                    relu(t[:], pl[:])
                    l0.append(t)
                out = []
                for h in range(2):
                    pl = ps.tile([P, ncols], FP32, tag="pmm")
                    mm(pl[:], [(w1_[0][:, h * P:(h + 1) * P], l0[0][:]),
                               (w1_[1][:, h * P:(h + 1) * P], l0[1][:])])
                    out.append(out_cb(h, pl))
                return out

            # ---- FP1: interp f3 (128 src) onto 256 targets, skip f2
            icm1 = []
            for h in range(2):
                pi = ps.tile([P, N2], FP32, tag="pmm")
                mm(pi[:], [(f3_pm[0][:, h * P:(h + 1) * P], w1t[:])])
                t = ac.tile([P, N2], FP32, tag=f"ih{h}")
                nc.vector.tensor_copy(t[:], pi[:])
                icm1.append(t)

            def ga_out(h, pl):
                t = pp.tile([P, N2], FP32, tag=f"ga{h}")
                relu(t[:], pl[:])
                return t

            ga_cm = fp_block(W["fp1_w0"], W["fp1_w1"], icm1, f2_cm, N2, ga_out)
            ga_pm = to_pm(ga_cm, N2, "gapm")

            # ---- FP2: interp ga (256 src) onto 512 targets, skip f1
            icm2 = []
            for h in range(2):
                pi = ps.tile([P, N1], FP32, tag="pmm")
                mm(pi[:], [(ga_pm[kc][:, h * P:(h + 1) * P], w2t[kc][:])
                           for kc in range(2)])
                t = ac.tile([P, N1], FP32, tag=f"ih{h}")
                nc.vector.tensor_copy(t[:], pi[:])
                icm2.append(t)

            def gb_out(h, pl):
                t = pp.tile([P, N1], FP32, tag=f"gb{h}")
                relu(t[:], pl[:])
                return t

            gb_cm = fp_block(W["fp2_w0"], W["fp2_w1"], icm2, f1_cm, N1, gb_out)
            gb_pm = to_pm(gb_cm, N1, "gbpm")

            # ---- FP3: interp gb (512 src) onto 20000 targets, skip f0 (1 ch)
            w0_, w1_ = W["fp3_w0"], W["fp3_w1"]
            col = 0
            n_strips = (N + 511) // 512
            for s in range(n_strips):
                ncols = min(512, N - col)
                w3c = []
                for kc in range(4):
                    t = io.tile([P, 512], FP32, tag=f"G{kc}")
                    nc.sync.dma_start(
                        t[:, :ncols], W3T_d[kc * P:(kc + 1) * P, col:col + ncols])
                    w3c.append(t)
                f0t = io.tile([1, 512], FP32, tag="f0t")
                nc.sync.dma_start(f0t[:, :ncols], f0T[:, col:col + ncols])
                icm3 = []
                for h in range(2):
                    pi = ps.tile([P, 512], FP32, tag="pmm")
                    mm(pi[:, :ncols],
                       [(gb_pm[kc][:, h * P:(h + 1) * P], w3c[kc][:, :ncols])
                        for kc in range(4)])
                    t = ac.tile([P, 512], FP32, tag=f"ih{h}")
                    nc.vector.tensor_copy(t[:, :ncols], pi[:, :ncols])
                    icm3.append(t)
                y0 = []
                for h in range(2):
                    pl = ps.tile([P, 512], FP32, tag="pmm")
                    mm(pl[:, :ncols],
                       [(w0_[0][:, h * P:(h + 1) * P], icm3[0][:, :ncols]),
                        (w0_[1][:, h * P:(h + 1) * P], icm3[1][:, :ncols]),
                        (w0_[2][:, h * P:(h + 1) * P], f0t[:, :ncols])])
                    t = ac.tile([P, 512], FP32, tag=f"fpl0_{h}")
                    relu(t[:, :ncols], pl[:, :ncols])
                    y0.append(t)
                for h in range(2):
                    pl = ps.tile([P, 512], FP32, tag="pmm")
                    mm(pl[:, :ncols],
                       [(w1_[0][:, h * P:(h + 1) * P], y0[0][:, :ncols]),
                        (w1_[1][:, h * P:(h + 1) * P], y0[1][:, :ncols])])
                    t = ac.tile([P, 512], FP32, tag=f"y1{h}")
                    relu(t[:, :ncols], pl[:, :ncols])
                    nc.sync.dma_start(
                        outT[h * P:(h + 1) * P, col:col + ncols], t[:, :ncols])
                col += ncols
    nc.compile()
    return nc


_NC_CACHE = {}


def kernel(**inputs):
    pc = np.asarray(inputs["pointcloud"], np.float32)
    wnames = list(WSPECS.keys())
    weights = {nm: np.ascontiguousarray(np.asarray(inputs[nm], np.float32))
               for nm in wnames}

    in_maps = [_host_maps(pc[b], weights) for b in range(B)]

    if "nc" not in _NC_CACHE:
        _NC_CACHE["nc"] = build_nc()
    nc = _NC_CACHE["nc"]

    r = run_bass_kernel_spmd(nc, in_maps, list(range(B)))
    if getattr(r, "exec_time_ns", None) is not None:
        print(f"HW exec time: {r.exec_time_ns} ns", flush=True)
    if getattr(r, "profile_json", None) is not None:
        print(f"profile_json: {r.profile_json}", flush=True)
    res = r.results
    out = np.stack([np.ascontiguousarray(res[b]["out_cm"].T) for b in range(B)])
    return out.astype(np.float32)


if __name__ == "__main__":
    rng = np.random.default_rng(0)
    fake = {"pointcloud": rng.standard_normal((B, N, 4), dtype=np.float32)}
    for nm, (r, c, _) in WSPECS.items():
        fake[nm] = rng.standard_normal((r, c), dtype=np.float32).astype(np.float32)
    o = kernel(**fake)
    print(o.shape, o.dtype)

